# revision 16
# baseline (speedup 1.0000x reference)
"""BCMGOOLSTM on 8 TRN2 NeuronCores — data-parallel over batch.

Strategy (hardcoded for T=1500, B=16, D=512, L=P=512, G=2048, 8 cores):
  - Shard batch: core i handles b in {2i, 2i+1} (B_loc=2).
  - Host prep: reconstruct block-circulant weights from the index tensors,
    fuse the output projection into the recurrence:
        u_t = h_{t-1} @ Wc + (x_t @ WihT + bias),  Wc = wym_w.T @ W_hh.T
    and permute gate columns to [f, c, i, o] for the on-device pipeline.
  - Device phases per core:
      1) w_out = xT.T @ WihT + bias  (bf16 matmul, PSUM->SBUF->DRAM bounce)
      2) sequential LSTM scan, 1500 steps, fully unrolled:
         4 gate matmuls/step (K=512 bf16) + identity-K2 matmul folding in
         w_out; sigmoid/tanh on ScalarE from PSUM; cell update on VectorE
         (fp32 cell state); h transposed back via 4 PE transposes into the
         [L-on-partitions] history buffer that feeds the next step's lhsT.
      3) ysT = wymT.T-style matmul over the whole h history -> f32 output.
  - Host post: ys[t, 2i+b, p] = ysT_i[p, 2t+b].

This file is self-contained (includes the walrus single-sync-wait workaround).
"""

import numpy as np
import ml_dtypes

# ---------------------------------------------------------------------------
# Problem constants (hardcoded per spec)
# ---------------------------------------------------------------------------
T, B, D = 1500, 16, 512
L = 512
P_DIM = 512
G = 4 * L          # 2048
KBLK = 16
NCORES = 8
BLOC = B // NCORES  # 2
TB = T * BLOC       # 3000
S_CHUNK = 8         # scan w_out chunk (steps per DMA)

_GATE_PERM = np.concatenate([
    np.arange(0, 512),        # f
    np.arange(1536, 2048),    # c
    np.arange(512, 1024),     # i
    np.arange(1024, 1536),    # o
]).astype(np.int64)

_BUILt = {}
DEBUG = False


# ---------------------------------------------------------------------------
# Walrus workaround: at most ONE semaphore wait per instruction
# ---------------------------------------------------------------------------
def _apply_tile_patches():
    import concourse.mybir as mybir
    import concourse.tile as tile_mod
    from concourse.vector_clock import ScopedClock

    def _drain_and_barrier(self, tick_clock, wait_clock):
        nc = self.nc
        drain_inst = nc.sync.drain()
        wait_clock.add_sem_waits(
            drain_inst.ins, ScopedClock({None: tick_clock.global_clock})
        )
        nc.all_engine_barrier()
        assert self.sems is not None
        popped = nc._tile_sem_poison_stack.pop()
        assert popped is self._sem_poison
        nc.clear_and_free_semaphores(list(self.sems.allocated().values()))
        nc.all_engine_barrier()

    tile_mod.TileContext._drain_and_barrier = _drain_and_barrier


def _fix_excess_waits(nc, max_waits=1):
    import concourse.mybir as mybir

    counter = 0
    for f in nc.m.functions:
        for blk in f.blocks:
            insts = list(blk.instructions)
            out = []
            changed = False
            for inst in insts:
                si = inst.sync_info
                if si is not None and len(si.on_wait) > max_waits:
                    waits = list(si.on_wait)
                    excess, keep = waits[:-max_waits], waits[-max_waits:]
                    for w in excess:
                        nop = mybir.InstNoOp(
                            name=f"waitspill-{counter}", ins=[], outs=[]
                        )
                        counter += 1
                        nop.engine = inst.engine
                        nop.sync_info = mybir.SyncInfo(on_wait=[w], on_update=[])
                        out.append(nop)
                    inst.sync_info = mybir.SyncInfo(
                        on_wait=keep, on_update=list(si.on_update)
                    )
                    changed = True
                out.append(inst)
            if changed:
                blk.instructions = out
    return counter


# ---------------------------------------------------------------------------
# Device kernel builder
# ---------------------------------------------------------------------------
def _build(scan_repeat=1):
    import concourse.bass as bass
    import concourse.mybir as mybir
    from concourse.tile import TileContext

    F32 = mybir.dt.float32
    BF16 = mybir.dt.bfloat16
    Sigmoid = mybir.ActivationFunctionType.Sigmoid
    Tanh = mybir.ActivationFunctionType.Tanh

    nc = bass.Bass(trn_type="TRN2")
    xT_d = nc.declare_dram_parameter("xT", [D, TB], F32, isOutput=False)
    wihT_d = nc.declare_dram_parameter("wihT", [D, G], F32, isOutput=False)
    wc_d = nc.declare_dram_parameter("wc", [L, G], F32, isOutput=False)
    wymT_d = nc.declare_dram_parameter("wymT", [L, P_DIM], F32, isOutput=False)
    bias_d = nc.declare_dram_parameter("bias", [1, G], F32, isOutput=False)
    ident_d = nc.declare_dram_parameter("ident", [2, 2], F32, isOutput=False)
    ysT_d = nc.declare_dram_parameter("ysT", [P_DIM, TB], F32, isOutput=True)
    if DEBUG:
        dbg_hist = nc.declare_dram_parameter("dbg_hist", [128, 4 * (T + 1) * BLOC], BF16, isOutput=True)
        dbg_woc = nc.declare_dram_parameter("dbg_woc", [BLOC, S_CHUNK * G], BF16, isOutput=True)

    n_mtiles = (TB + 127) // 128  # 24 (last has 56 rows)

    with TileContext(nc) as tc:
        with tc.tile_pool(name="persist", bufs=1) as pp, \
             tc.tile_pool(name="dram", bufs=1, space="DRAM") as dp:

            # persistent SBUF tensors
            xT_sb = pp.tile([128, 4, TB], BF16)
            wihT_sb = pp.tile([128, 4, G], BF16)
            wc_sb = pp.tile([128, 4, G], BF16)
            wymT_sb = pp.tile([128, 4, P_DIM], BF16)
            bias_sb = pp.tile([1, G], BF16)
            ones_sb = pp.tile([1, 128], BF16)
            ident = pp.tile([2, 2], BF16)
            ht_hist = pp.tile([128, 4, T + 1, BLOC], BF16)

            nc.gpsimd.dma_start(xT_sb[:], xT_d.rearrange("(k p) n -> p k n", p=128))
            nc.gpsimd.dma_start(wihT_sb[:], wihT_d.rearrange("(k p) g -> p k g", p=128))
            nc.gpsimd.dma_start(wc_sb[:], wc_d.rearrange("(k p) g -> p k g", p=128))
            nc.gpsimd.dma_start(wymT_sb[:], wymT_d.rearrange("(k p) g -> p k g", p=128))
            nc.gpsimd.dma_start(bias_sb[:], bias_d[:])
            nc.gpsimd.dma_start(ident[:], ident_d[:])
            nc.vector.memset(ones_sb[:], 1.0)
            nc.vector.memset(ht_hist[:, :, 0, :], 0.0)

            wout_i = dp.tile([T * BLOC, G], BF16)

            # ---------------- phase 1: w_out ----------------
            p1_dmas = []
            with tc.tile_pool(name="p1sb", bufs=3) as p1, \
                 tc.tile_pool(name="p1ps", bufs=3, space="PSUM") as p1p:
                for m in range(n_mtiles):
                    rows = min(128, TB - m * 128)
                    wtile = p1.tile([128, G], BF16, tag="wtile")
                    for nchunk in range(4):
                        pw = p1p.tile([128, 512], F32, tag="pw")
                        for k in range(4):
                            nc.tensor.matmul(
                                pw[:rows],
                                xT_sb[:, k, m * 128 : m * 128 + rows],
                                wihT_sb[:, k, nchunk * 512 : (nchunk + 1) * 512],
                                start=(k == 0),
                                stop=False,
                            )
                        nc.tensor.matmul(
                            pw[:rows],
                            ones_sb[:, :rows],
                            bias_sb[:, nchunk * 512 : (nchunk + 1) * 512],
                            start=False,
                            stop=True,
                        )
                        nc.vector.tensor_copy(
                            wtile[:rows, nchunk * 512 : (nchunk + 1) * 512],
                            pw[:rows],
                        )
                    # scatter to DRAM: partition p = 2*t' + b -> wout_i[b, 64m+t', :]
                    p1_dmas.append(
                        nc.sync.dma_start(
                            wout_i[m * 128 : m * 128 + rows, :],
                            wtile[:rows],
                        )
                    )

            # ---------------- phase 2: scan (V10) ----------------
            # Transposed elementwise chain; w_out injected via K=2 identity
            # matmuls emitted one step ahead so they fill the PE tail; the
            # PE-transpose outputs share the gate psum tags (8 banks total).
            ORDER = (1, 0, 2, 3)  # c, f, i, o
            NM = {0: "sig_f", 1: "tanh_c", 2: "sig_i", 3: "sig_o"}
            with tc.tile_pool(name="wop", bufs=2) as wp, \
                 tc.tile_pool(name="gsb", bufs=2) as gp, \
                 tc.tile_pool(name="state", bufs=1) as stp, \
                 tc.tile_pool(name="gps", bufs=2, space="PSUM") as gps:

                ctT_a = stp.tile([128, 4, BLOC], F32)
                ctT_b = stp.tile([128, 4, BLOC], F32)
                nc.vector.memset(ctT_a[:], 0.0)
                nc.vector.memset(ctT_b[:], 0.0)
                cts = [ctT_a, ctT_b]
                v3 = lambda ap: ap.rearrange("p (k b) -> p k b", k=4)

                from concourse.tile import add_dep_helper

                def dma_chunk(cidx):
                    woc = wp.tile([BLOC, S_CHUNK * G], BF16, tag="wo")
                    n_here = min(S_CHUNK, T - cidx * S_CHUNK)
                    rd = nc.sync.dma_start(
                        woc[:, : n_here * G].rearrange("b (t g) -> b t g", g=G),
                        wout_i[
                            cidx * S_CHUNK * 2 : (cidx * S_CHUNK + n_here) * 2, :
                        ].rearrange("(t b) g -> b t g", b=2),
                    )
                    # RAW through DRAM isn't tile-tracked: order the chunk
                    # read after the phase-1 write that produced it.
                    m = (cidx * S_CHUNK) // 64
                    m2 = (cidx * S_CHUNK + S_CHUNK - 1) // 64
                    add_dep_helper(rd.ins, p1_dmas[m].ins, reason="wout RAW")
                    if m2 != m and m2 < len(p1_dmas):
                        add_dep_helper(rd.ins, p1_dmas[m2].ins, reason="wout RAW2")
                    if DEBUG and cidx == 0:
                        nc.sync.dma_start(dbg_woc[:], woc[:])
                    return woc

                def emit_k2s(tt, woc):
                    s = tt % S_CHUNK
                    pgs = {}
                    for gid in ORDER:
                        pg = gps.tile([BLOC, 512], F32, tag=f"g{gid}")
                        nc.tensor.matmul(
                            pg[:],
                            ident[:],
                            woc[:, s * G + gid * 512 : s * G + (gid + 1) * 512],
                            start=True,
                            stop=False,
                        )
                        pgs[gid] = pg
                    return pgs

                wo = dma_chunk(0)
                pgs = emit_k2s(0, wo)

                for t in range(T):
                    ctT_prev, ctT_new = cts[t % 2], cts[(t + 1) % 2]
                    sigs = {}
                    pts = {}
                    for j, gid in enumerate(ORDER):
                        pg = pgs[gid]
                        for k in range(4):
                            nc.tensor.matmul(
                                pg[:],
                                ht_hist[:, k, t, :],
                                wc_sb[:, k, gid * 512 : (gid + 1) * 512],
                                start=False,
                                stop=(k == 3),
                            )
                        sg = gp.tile([BLOC, 512], BF16, tag=NM[gid])
                        nc.scalar.activation(
                            sg[:], pg[:], Tanh if gid == 1 else Sigmoid
                        )
                        sigs[gid] = sg
                        if j == 2:
                            # transpose tanh_c while PE still streams gate o
                            pt1 = gps.tile([128, 8], BF16, tag="g1")
                            for k in range(4):
                                nc.tensor.transpose(
                                    pt1[:, 2 * k : 2 * k + 2],
                                    sigs[1][:, 128 * k : 128 * (k + 1)],
                                    ident[:],
                                )
                            pts[1] = pt1

                    for gid in (0, 2):  # f, i
                        pt = gps.tile([128, 8], BF16, tag=f"g{gid}")
                        for k in range(4):
                            nc.tensor.transpose(
                                pt[:, 2 * k : 2 * k + 2],
                                sigs[gid][:, 128 * k : 128 * (k + 1)],
                                ident[:],
                            )
                        pts[gid] = pt

                    # K2s for step t+1 fill the PE tail
                    if t + 1 < T:
                        if (t + 1) % S_CHUNK == 0:
                            wo = dma_chunk((t + 1) // S_CHUNK)
                        pgs = emit_k2s(t + 1, wo)

                    pt3 = gps.tile([128, 8], BF16, tag="g3")
                    for k in range(4):
                        nc.tensor.transpose(
                            pt3[:, 2 * k : 2 * k + 2],
                            sigs[3][:, 128 * k : 128 * (k + 1)],
                            ident[:],
                        )
                    pts[3] = pt3

                    tcT = gp.tile([128, 4, BLOC], BF16, tag="tcT")
                    nc.vector.tensor_copy(tcT[:], v3(pts[1][:]))
                    t2T = gp.tile([128, 4, BLOC], F32, tag="t2T")
                    nc.vector.tensor_tensor(
                        t2T[:], v3(pts[0][:]), ctT_prev[:], mybir.AluOpType.mult
                    )
                    t1T = gp.tile([128, 4, BLOC], BF16, tag="t1T")
                    nc.vector.tensor_tensor(
                        t1T[:], v3(pts[2][:]), tcT[:], mybir.AluOpType.mult
                    )
                    nc.vector.tensor_tensor(
                        ctT_new[:], t1T[:], t2T[:], mybir.AluOpType.add
                    )
                    tanh_ctT = gp.tile([128, 4, BLOC], BF16, tag="tanh_ctT")
                    nc.scalar.activation(tanh_ctT[:], ctT_new[:], Tanh)
                    nc.vector.tensor_tensor(
                        ht_hist[:, :, t + 1, :],
                        v3(pts[3][:]),
                        tanh_ctT[:],
                        mybir.AluOpType.mult,
                    )

            if DEBUG:
                nc.sync.dma_start(
                    dbg_hist[:], ht_hist[:].rearrange("p k t b -> p (k t b)")
                )

            # ---------------- phase 3: ys ----------------
            with tc.tile_pool(name="p3sb", bufs=3) as p3, \
                 tc.tile_pool(name="p3ps", bufs=3, space="PSUM") as p3p:
                NT = 512
                n_nt = (TB + NT - 1) // NT  # 6 (last 440)
                for m in range(4):
                    for nt in range(n_nt):
                        cols = min(NT, TB - nt * NT)
                        py = p3p.tile([128, NT], F32, tag="py")
                        for k in range(4):
                            nc.tensor.matmul(
                                py[:, :cols],
                                wymT_sb[:, k, m * 128 : (m + 1) * 128],
                                ht_hist[:, k, :, :].rearrange("p t b -> p (t b)")[
                                    :, 2 + nt * NT : 2 + nt * NT + cols
                                ],
                                start=(k == 0),
                                stop=(k == 3),
                            )
                        ytile = p3.tile([128, NT], F32, tag="ytile")
                        nc.vector.tensor_copy(ytile[:, :cols], py[:, :cols])
                        nc.sync.dma_start(
                            ysT_d.rearrange("(mm p) n -> mm p n", p=128)[
                                m, :, nt * NT : nt * NT + cols
                            ],
                            ytile[:, :cols],
                        )

    _fix_excess_waits(nc)
    return nc


def _get_nc(scan_repeat=1):
    key = scan_repeat
    if key not in _BUILt:
        _apply_tile_patches()
        _BUILt[key] = _build(scan_repeat)
    return _BUILt[key]


# ---------------------------------------------------------------------------
# Host entry point
# ---------------------------------------------------------------------------
def kernel(x, vector_ih, vector_hh, bias_ih, wym_w, indx_ih, indx_hh):
    from concourse.bass_utils import run_bass_kernel_spmd

    x = np.asarray(x, dtype=np.float32)
    vector_ih = np.asarray(vector_ih, dtype=np.float32)
    vector_hh = np.asarray(vector_hh, dtype=np.float32)
    bias_ih = np.asarray(bias_ih, dtype=np.float32)
    wym_w = np.asarray(wym_w, dtype=np.float32)
    indx_ih = np.asarray(indx_ih)
    indx_hh = np.asarray(indx_hh)

    # reconstruct weights (host-side layout prep)
    wihT = vector_ih[indx_ih.reshape(-1).astype(np.int64)].reshape(D, G)  # [D, G]
    whh = vector_hh[indx_hh.reshape(-1).astype(np.int64)].reshape(P_DIM, G)  # [P, G]
    wc = (wym_w.T.astype(np.float64) @ whh.astype(np.float64)).astype(np.float32)

    wihT = np.ascontiguousarray(wihT[:, _GATE_PERM])
    wc = np.ascontiguousarray(wc[:, _GATE_PERM])
    bias = np.ascontiguousarray(bias_ih[_GATE_PERM]).reshape(1, G)
    wymT = np.ascontiguousarray(wym_w.T)
    ident = np.eye(2, dtype=np.float32)

    nc = _get_nc()
    in_maps = []
    for i in range(NCORES):
        x_loc = x[:, 2 * i : 2 * i + 2, :].reshape(TB, D)
        xT = np.ascontiguousarray(x_loc.T)
        in_maps.append({
            "xT": xT,
            "wihT": wihT,
            "wc": wc,
            "wymT": wymT,
            "bias": bias,
            "ident": ident,
        })

    res = run_bass_kernel_spmd(nc, in_maps, core_ids=list(range(NCORES)))
    globals()["_LAST_RES"] = res

    out = np.empty((T, B, P_DIM), dtype=np.float32)
    for i in range(NCORES):
        ysT = res.results[i]["ysT"]  # [P, TB]
        ys_loc = ysT.T.reshape(T, BLOC, P_DIM)
        out[:, 2 * i : 2 * i + 2, :] = ys_loc
    return out


# revision 18
# speedup vs baseline: 203.2542x; 203.2542x over previous
"""BCMGOOLSTM on 8 TRN2 NeuronCores — data-parallel over batch.

Strategy (hardcoded for T=1500, B=16, D=512, L=P=512, G=2048, 8 cores):
  - Shard batch: core i handles b in {2i, 2i+1} (B_loc=2).
  - Host prep: reconstruct block-circulant weights from the index tensors,
    fuse the output projection into the recurrence:
        u_t = h_{t-1} @ Wc + (x_t @ WihT + bias),  Wc = wym_w.T @ W_hh.T
    and permute gate columns to [f, c, i, o] for the on-device pipeline.
  - Device phases per core:
      1) w_out = xT.T @ WihT + bias  (bf16 matmul, PSUM->SBUF->DRAM bounce)
      2) sequential LSTM scan, 1500 steps, fully unrolled:
         4 gate matmuls/step (K=512 bf16) + identity-K2 matmul folding in
         w_out; sigmoid/tanh on ScalarE from PSUM; cell update on VectorE
         (fp32 cell state); h transposed back via 4 PE transposes into the
         [L-on-partitions] history buffer that feeds the next step's lhsT.
      3) ysT = wymT.T-style matmul over the whole h history -> f32 output.
  - Host post: ys[t, 2i+b, p] = ysT_i[p, 2t+b].

This file is self-contained (includes the walrus single-sync-wait workaround).
"""

import numpy as np
import ml_dtypes

# ---------------------------------------------------------------------------
# Problem constants (hardcoded per spec)
# ---------------------------------------------------------------------------
T, B, D = 1500, 16, 512
L = 512
P_DIM = 512
G = 4 * L          # 2048
KBLK = 16
NCORES = 8
BLOC = B // NCORES  # 2
TB = T * BLOC       # 3000
S_CHUNK = 8         # scan w_out chunk (steps per DMA)

_GATE_PERM = np.concatenate([
    np.arange(0, 512),        # f
    np.arange(1536, 2048),    # c
    np.arange(512, 1024),     # i
    np.arange(1024, 1536),    # o
]).astype(np.int64)

_BUILt = {}
DEBUG = False


# ---------------------------------------------------------------------------
# Walrus workaround: at most ONE semaphore wait per instruction
# ---------------------------------------------------------------------------
def _apply_tile_patches():
    import concourse.mybir as mybir
    import concourse.tile as tile_mod
    from concourse.vector_clock import ScopedClock

    def _drain_and_barrier(self, tick_clock, wait_clock):
        nc = self.nc
        drain_inst = nc.sync.drain()
        wait_clock.add_sem_waits(
            drain_inst.ins, ScopedClock({None: tick_clock.global_clock})
        )
        nc.all_engine_barrier()
        assert self.sems is not None
        popped = nc._tile_sem_poison_stack.pop()
        assert popped is self._sem_poison
        nc.clear_and_free_semaphores(list(self.sems.allocated().values()))
        nc.all_engine_barrier()

    tile_mod.TileContext._drain_and_barrier = _drain_and_barrier


def _fix_excess_waits(nc, max_waits=1):
    import concourse.mybir as mybir

    counter = 0
    for f in nc.m.functions:
        for blk in f.blocks:
            insts = list(blk.instructions)
            out = []
            changed = False
            for inst in insts:
                si = inst.sync_info
                if si is not None and len(si.on_wait) > max_waits:
                    waits = list(si.on_wait)
                    excess, keep = waits[:-max_waits], waits[-max_waits:]
                    for w in excess:
                        nop = mybir.InstNoOp(
                            name=f"waitspill-{counter}", ins=[], outs=[]
                        )
                        counter += 1
                        nop.engine = inst.engine
                        nop.sync_info = mybir.SyncInfo(on_wait=[w], on_update=[])
                        out.append(nop)
                    inst.sync_info = mybir.SyncInfo(
                        on_wait=keep, on_update=list(si.on_update)
                    )
                    changed = True
                out.append(inst)
            if changed:
                blk.instructions = out
    return counter


# ---------------------------------------------------------------------------
# Device kernel builder
# ---------------------------------------------------------------------------
def _build(scan_repeat=1):
    import concourse.bass as bass
    import concourse.mybir as mybir
    from concourse.tile import TileContext

    F32 = mybir.dt.float32
    BF16 = mybir.dt.bfloat16
    Sigmoid = mybir.ActivationFunctionType.Sigmoid
    Tanh = mybir.ActivationFunctionType.Tanh

    nc = bass.Bass(trn_type="TRN2")
    xT_d = nc.declare_dram_parameter("xT", [D, TB], F32, isOutput=False)
    wihT_d = nc.declare_dram_parameter("wihT", [D, G], F32, isOutput=False)
    wc_d = nc.declare_dram_parameter("wc", [L, G], F32, isOutput=False)
    wymT_d = nc.declare_dram_parameter("wymT", [L, P_DIM], F32, isOutput=False)
    bias_d = nc.declare_dram_parameter("bias", [1, G], F32, isOutput=False)
    ident_d = nc.declare_dram_parameter("ident", [2, 2], F32, isOutput=False)
    ysT_d = nc.declare_dram_parameter("ysT", [P_DIM, TB], F32, isOutput=True)
    if DEBUG:
        dbg_hist = nc.declare_dram_parameter("dbg_hist", [128, 4 * (T + 1) * BLOC], BF16, isOutput=True)
        dbg_woc = nc.declare_dram_parameter("dbg_woc", [BLOC, S_CHUNK * G], BF16, isOutput=True)

    n_mtiles = (TB + 127) // 128  # 24 (last has 56 rows)

    with TileContext(nc) as tc:
        with tc.tile_pool(name="persist", bufs=1) as pp, \
             tc.tile_pool(name="dram", bufs=1, space="DRAM") as dp:

            # persistent SBUF tensors
            xT_sb = pp.tile([128, 4, TB], BF16)
            wihT_sb = pp.tile([128, 4, G], BF16)
            wc_sb = pp.tile([128, 4, G], BF16)
            wymT_sb = pp.tile([128, 4, P_DIM], BF16)
            bias_sb = pp.tile([1, G], BF16)
            ones_sb = pp.tile([1, 128], BF16)
            ident = pp.tile([2, 2], BF16)
            ht_hist = pp.tile([128, 4, T + 1, BLOC], BF16)

            nc.gpsimd.dma_start(xT_sb[:], xT_d.rearrange("(k p) n -> p k n", p=128))
            nc.gpsimd.dma_start(wihT_sb[:], wihT_d.rearrange("(k p) g -> p k g", p=128))
            nc.gpsimd.dma_start(wc_sb[:], wc_d.rearrange("(k p) g -> p k g", p=128))
            nc.gpsimd.dma_start(wymT_sb[:], wymT_d.rearrange("(k p) g -> p k g", p=128))
            nc.gpsimd.dma_start(bias_sb[:], bias_d[:])
            nc.gpsimd.dma_start(ident[:], ident_d[:])
            nc.vector.memset(ones_sb[:], 1.0)
            nc.vector.memset(ht_hist[:, :, 0, :], 0.0)

            wout_i = dp.tile([T * BLOC, G], BF16)

            # ---------------- phase 1: w_out ----------------
            p1_dmas = []
            with tc.tile_pool(name="p1sb", bufs=3) as p1, \
                 tc.tile_pool(name="p1ps", bufs=3, space="PSUM") as p1p:
                for m in range(n_mtiles):
                    rows = min(128, TB - m * 128)
                    wtile = p1.tile([128, G], BF16, tag="wtile")
                    for nchunk in range(4):
                        pw = p1p.tile([128, 512], F32, tag="pw")
                        for k in range(4):
                            nc.tensor.matmul(
                                pw[:rows],
                                xT_sb[:, k, m * 128 : m * 128 + rows],
                                wihT_sb[:, k, nchunk * 512 : (nchunk + 1) * 512],
                                start=(k == 0),
                                stop=False,
                            )
                        nc.tensor.matmul(
                            pw[:rows],
                            ones_sb[:, :rows],
                            bias_sb[:, nchunk * 512 : (nchunk + 1) * 512],
                            start=False,
                            stop=True,
                        )
                        nc.vector.tensor_copy(
                            wtile[:rows, nchunk * 512 : (nchunk + 1) * 512],
                            pw[:rows],
                        )
                    # scatter to DRAM: partition p = 2*t' + b -> wout_i[b, 64m+t', :]
                    p1_dmas.append(
                        nc.sync.dma_start(
                            wout_i[m * 128 : m * 128 + rows, :],
                            wtile[:rows],
                        )
                    )

            # ---------------- phase 2: scan (V10) ----------------
            # Transposed elementwise chain; w_out injected via K=2 identity
            # matmuls emitted one step ahead so they fill the PE tail; the
            # PE-transpose outputs share the gate psum tags (8 banks total).
            ORDER = (1, 0, 2, 3)  # c, f, i, o
            NM = {0: "sig_f", 1: "tanh_c", 2: "sig_i", 3: "sig_o"}
            with tc.tile_pool(name="wop", bufs=2) as wp, \
                 tc.tile_pool(name="gsb", bufs=2) as gp, \
                 tc.tile_pool(name="state", bufs=1) as stp, \
                 tc.tile_pool(name="gps", bufs=2, space="PSUM") as gps:

                ctT_a = stp.tile([128, 4, BLOC], F32)
                ctT_b = stp.tile([128, 4, BLOC], F32)
                nc.vector.memset(ctT_a[:], 0.0)
                nc.vector.memset(ctT_b[:], 0.0)
                cts = [ctT_a, ctT_b]
                v3 = lambda ap: ap.rearrange("p (k b) -> p k b", k=4)

                from concourse.tile import add_dep_helper

                def dma_chunk(cidx):
                    woc = wp.tile([BLOC, S_CHUNK * G], BF16, tag="wo")
                    n_here = min(S_CHUNK, T - cidx * S_CHUNK)
                    rd = nc.sync.dma_start(
                        woc[:, : n_here * G].rearrange("b (t g) -> b t g", g=G),
                        wout_i[
                            cidx * S_CHUNK * 2 : (cidx * S_CHUNK + n_here) * 2, :
                        ].rearrange("(t b) g -> b t g", b=2),
                    )
                    # RAW through DRAM isn't tile-tracked: order the chunk
                    # read after the phase-1 write that produced it.
                    m = (cidx * S_CHUNK) // 64
                    m2 = (cidx * S_CHUNK + S_CHUNK - 1) // 64
                    add_dep_helper(rd.ins, p1_dmas[m].ins, reason="wout RAW")
                    if m2 != m and m2 < len(p1_dmas):
                        add_dep_helper(rd.ins, p1_dmas[m2].ins, reason="wout RAW2")
                    if DEBUG and cidx == 0:
                        nc.sync.dma_start(dbg_woc[:], woc[:])
                    return woc

                def emit_k2s(tt, woc):
                    s = tt % S_CHUNK
                    pgs = {}
                    for gid in ORDER:
                        pg = gps.tile([BLOC, 512], F32, tag=f"g{gid}")
                        nc.tensor.matmul(
                            pg[:],
                            ident[:],
                            woc[:, s * G + gid * 512 : s * G + (gid + 1) * 512],
                            start=True,
                            stop=False,
                        )
                        pgs[gid] = pg
                    return pgs

                wo = dma_chunk(0)
                pgs = emit_k2s(0, wo)

                for t in range(T):
                    ctT_prev, ctT_new = cts[t % 2], cts[(t + 1) % 2]
                    sigs = {}
                    pts = {}
                    for j, gid in enumerate(ORDER):
                        pg = pgs[gid]
                        for k in range(4):
                            nc.tensor.matmul(
                                pg[:],
                                ht_hist[:, k, t, :],
                                wc_sb[:, k, gid * 512 : (gid + 1) * 512],
                                start=False,
                                stop=(k == 3),
                            )
                        sg = gp.tile([BLOC, 512], BF16, tag=NM[gid])
                        nc.scalar.activation(
                            sg[:], pg[:], Tanh if gid == 1 else Sigmoid
                        )
                        sigs[gid] = sg
                        if j == 2:
                            # transpose tanh_c while PE still streams gate o
                            pt1 = gps.tile([128, 8], BF16, tag="g1")
                            for k in range(4):
                                nc.tensor.transpose(
                                    pt1[:, 2 * k : 2 * k + 2],
                                    sigs[1][:, 128 * k : 128 * (k + 1)],
                                    ident[:],
                                )
                            pts[1] = pt1

                    for gid in (0, 2):  # f, i
                        pt = gps.tile([128, 8], BF16, tag=f"g{gid}")
                        for k in range(4):
                            nc.tensor.transpose(
                                pt[:, 2 * k : 2 * k + 2],
                                sigs[gid][:, 128 * k : 128 * (k + 1)],
                                ident[:],
                            )
                        pts[gid] = pt

                    # K2s for step t+1 fill the PE tail
                    if t + 1 < T:
                        if (t + 1) % S_CHUNK == 0:
                            wo = dma_chunk((t + 1) // S_CHUNK)
                        pgs = emit_k2s(t + 1, wo)

                    pt3 = gps.tile([128, 8], BF16, tag="g3")
                    for k in range(4):
                        nc.tensor.transpose(
                            pt3[:, 2 * k : 2 * k + 2],
                            sigs[3][:, 128 * k : 128 * (k + 1)],
                            ident[:],
                        )
                    pts[3] = pt3

                    tcT = gp.tile([128, 4, BLOC], BF16, tag="tcT")
                    nc.vector.tensor_copy(tcT[:], v3(pts[1][:]))
                    t2T = gp.tile([128, 4, BLOC], F32, tag="t2T")
                    nc.vector.tensor_tensor(
                        t2T[:], v3(pts[0][:]), ctT_prev[:], mybir.AluOpType.mult
                    )
                    t1T = gp.tile([128, 4, BLOC], BF16, tag="t1T")
                    nc.vector.tensor_tensor(
                        t1T[:], v3(pts[2][:]), tcT[:], mybir.AluOpType.mult
                    )
                    nc.vector.tensor_tensor(
                        ctT_new[:], t1T[:], t2T[:], mybir.AluOpType.add
                    )
                    tanh_ctT = gp.tile([128, 4, BLOC], BF16, tag="tanh_ctT")
                    nc.scalar.activation(tanh_ctT[:], ctT_new[:], Tanh)
                    nc.vector.tensor_tensor(
                        ht_hist[:, :, t + 1, :],
                        v3(pts[3][:]),
                        tanh_ctT[:],
                        mybir.AluOpType.mult,
                    )

            if DEBUG:
                nc.sync.dma_start(
                    dbg_hist[:], ht_hist[:].rearrange("p k t b -> p (k t b)")
                )

            # ---------------- phase 3: ys ----------------
            with tc.tile_pool(name="p3sb", bufs=3) as p3, \
                 tc.tile_pool(name="p3ps", bufs=3, space="PSUM") as p3p:
                NT = 512
                n_nt = (TB + NT - 1) // NT  # 6 (last 440)
                for m in range(4):
                    for nt in range(n_nt):
                        cols = min(NT, TB - nt * NT)
                        py = p3p.tile([128, NT], F32, tag="py")
                        for k in range(4):
                            nc.tensor.matmul(
                                py[:, :cols],
                                wymT_sb[:, k, m * 128 : (m + 1) * 128],
                                ht_hist[:, k, :, :].rearrange("p t b -> p (t b)")[
                                    :, 2 + nt * NT : 2 + nt * NT + cols
                                ],
                                start=(k == 0),
                                stop=(k == 3),
                            )
                        ytile = p3.tile([128, NT], F32, tag="ytile")
                        nc.vector.tensor_copy(ytile[:, :cols], py[:, :cols])
                        nc.sync.dma_start(
                            ysT_d.rearrange("(mm p) n -> mm p n", p=128)[
                                m, :, nt * NT : nt * NT + cols
                            ],
                            ytile[:, :cols],
                        )

    _fix_excess_waits(nc)
    return nc


def _get_nc(scan_repeat=1):
    key = scan_repeat
    if key not in _BUILt:
        _apply_tile_patches()
        _BUILt[key] = _build(scan_repeat)
    return _BUILt[key]


# ---------------------------------------------------------------------------
# Host entry point
# ---------------------------------------------------------------------------
def _prep_in_maps(x, vector_ih, vector_hh, bias_ih, wym_w, indx_ih, indx_hh):
    x = np.asarray(x, dtype=np.float32)
    vector_ih = np.asarray(vector_ih, dtype=np.float32)
    vector_hh = np.asarray(vector_hh, dtype=np.float32)
    bias_ih = np.asarray(bias_ih, dtype=np.float32)
    wym_w = np.asarray(wym_w, dtype=np.float32)
    indx_ih = np.asarray(indx_ih)
    indx_hh = np.asarray(indx_hh)

    # reconstruct weights (host-side layout prep)
    wihT = vector_ih[indx_ih.reshape(-1).astype(np.int64)].reshape(D, G)  # [D, G]
    whh = vector_hh[indx_hh.reshape(-1).astype(np.int64)].reshape(P_DIM, G)  # [P, G]
    wc = (wym_w.T.astype(np.float64) @ whh.astype(np.float64)).astype(np.float32)

    wihT = np.ascontiguousarray(wihT[:, _GATE_PERM])
    wc = np.ascontiguousarray(wc[:, _GATE_PERM])
    bias = np.ascontiguousarray(bias_ih[_GATE_PERM]).reshape(1, G)
    wymT = np.ascontiguousarray(wym_w.T)
    ident = np.eye(2, dtype=np.float32)

    in_maps = []
    for i in range(NCORES):
        x_loc = x[:, 2 * i : 2 * i + 2, :].reshape(TB, D)
        xT = np.ascontiguousarray(x_loc.T)
        in_maps.append({
            "xT": xT,
            "wihT": wihT,
            "wc": wc,
            "wymT": wymT,
            "bias": bias,
            "ident": ident,
        })
    return in_maps


def kernel(x, vector_ih, vector_hh, bias_ih, wym_w, indx_ih, indx_hh):
    from concourse.bass_utils import run_bass_kernel_spmd

    in_maps = _prep_in_maps(
        x, vector_ih, vector_hh, bias_ih, wym_w, indx_ih, indx_hh
    )
    nc = _get_nc()
    res = run_bass_kernel_spmd(nc, in_maps, core_ids=list(range(NCORES)))
    globals()["_LAST_RES"] = res

    out = np.empty((T, B, P_DIM), dtype=np.float32)
    for i in range(NCORES):
        ysT = res.results[i]["ysT"]  # [P, TB]
        ys_loc = ysT.T.reshape(T, BLOC, P_DIM)
        out[:, 2 * i : 2 * i + 2, :] = ys_loc
    return out
